# revision 14
# baseline (speedup 1.0000x reference)
"""ANI-style MoE routing kernel for 8 Trainium2 NeuronCores — v2.

Strategy (data-parallel + host routing):
  - Host: sort atoms by type, split each type's list evenly across 8 cores,
    build per-core per-expert contiguous batches padded to per-expert
    capacities (multiples of 1024).  Batches are feature-major bf16
    [384, sum(caps)].  Small overflows (<= SHED_MAX atoms) are computed
    exactly on the host in f64 instead of paying a whole device block.
  - Device (per core), per 1024-atom block, dim-major:
      mm1 (PE): z1 = W1^T x.  H1=192 -> m0 [128,1024] + m1 [64,1024]
        folded to [128,512] (two 512-atom column halves stacked on
        partitions 0/64) so pointwise ops touch zero garbage lanes.
      celu1: t1 = exp(z1) [ACT]; u1 = min(t1-1,0) [DVE ts 2x]; g1 =
        max(z1,u1) -> bf16 [DVE tt].
      mm2 (PE): z2 = W2^T g1.  H2=160 -> m0 [128,1024] + m1 [32,1024]
        folded to [64,512] (quarters at partition 0/32 x column halves).
      layer 3 is linear and the final output is a scalar, so NO mm3:
        per-dim sums of celu(z2) suffice.  celu(z2) = max(z2, u2) with
        u2 = min(exp(z2)-1, 0), so one DVE scalar_tensor_tensor /
        tensor_tensor_reduce pass with accum_out yields the per-block
        per-dim sums directly.  The u2 helper runs on ACT as
        relu(1-exp(z2)) (= -u2, sign folded into the STT) for m0 and on
        DVE for m1, balancing engine load.  Sum columns land in an SBUF
        tile, DMA'd out once at the end.
  - Host: S_e[dim] = sum of block columns; energy = sum_e w3[e]^T S_e in
    f64, + b3*counts + shed energies.

Zero-bias (always true for this problem's init) makes padding rows
self-cancelling: z=0 -> celu contribution exactly 0.  The general-bias
path adds per-layer bias matmuls and corrects pads on the host.
"""

import os
import sys

import numpy as np

try:
    import concourse.bass as bass  # noqa: F401
except ImportError:  # pragma: no cover
    sys.path.insert(0, "/opt/trn_rl_repo")
    import concourse.bass as bass  # noqa: F401

import concourse.mybir as mybir
import concourse.tile as tile
from concourse import bacc
from concourse import bass_utils

IN_DIM = 384
H1 = 192
H2 = 160
E = 4
N_CORES = 8
N_ATOMS = 262144

BLOCK = 1024
HB = 512  # half block

F32 = mybir.dt.float32
BF16 = mybir.dt.bfloat16
AF = mybir.ActivationFunctionType
ALU = mybir.AluOpType

# engine assignment knobs (A/B-testable)
UN2M0 = os.environ.get("UN2M0", "act")  # act | dve
U2M1 = os.environ.get("U2M1", "dve")  # dve | pool
U1_POOL = os.environ.get("U1_POOL", "0") == "1"


def _build_graph(with_bias: bool, caps, repeat: int = 1):
    nc = bacc.Bacc(
        "TRN2",
        target_bir_lowering=False,
        debug=False,
        enable_asserts=False,
        num_devices=N_CORES,
    )
    total_cap = sum(caps)
    nb = total_cap // BLOCK
    xT = nc.dram_tensor("xT", [IN_DIM, total_cap], BF16, kind="ExternalInput").ap()
    W1 = nc.dram_tensor("W1", [E, IN_DIM, H1], BF16, kind="ExternalInput").ap()
    W2 = nc.dram_tensor("W2", [E, H1, H2], BF16, kind="ExternalInput").ap()
    B1 = B2 = None
    if with_bias:
        B1 = nc.dram_tensor("B1", [E, H1], BF16, kind="ExternalInput").ap()
        B2 = nc.dram_tensor("B2", [E, H2], BF16, kind="ExternalInput").ap()
    outS = nc.dram_tensor("outS", [128, 2 * nb], F32, kind="ExternalOutput").ap()

    with tile.TileContext(nc) as tc:
        _emit(tc, xT, W1, W2, B1, B2, outS, with_bias, caps, repeat)
    nc.compile()
    return nc


def _emit(tc, xT, W1, W2, B1, B2, outS, with_bias, caps, repeat=1):
    import contextlib

    nc = tc.nc
    xT3 = xT.rearrange("(kt kp) n -> kp kt n", kp=128)  # [128, 3, total]
    nb = sum(caps) // BLOCK

    with (
        tc.tile_pool(name="wpool", bufs=1) as wp,
        tc.tile_pool(name="xpool", bufs=int(os.environ.get("X_BUFS", "3"))) as xp,
        tc.tile_pool(name="t1pool", bufs=2) as t1p,
        tc.tile_pool(name="u1pool", bufs=2) as u1p,
        tc.tile_pool(name="g1pool", bufs=2) as g1p,
        tc.tile_pool(name="t2pool", bufs=2) as t2p,
        tc.tile_pool(name="u2pool", bufs=2) as u2p,
        tc.tile_pool(name="cpool", bufs=1) as cp,
        tc.tile_pool(name="spool", bufs=1) as sp,
        tc.tile_pool(name="z1m0p", bufs=2, space="PSUM") as z1m0p,
        tc.tile_pool(name="z1m1p", bufs=1, space="PSUM") as z1m1p,
        tc.tile_pool(name="z2m0p", bufs=1, space="PSUM") as z2m0p,
        tc.tile_pool(name="z2m1p", bufs=1, space="PSUM") as z2m1p,
    ):
        # ---- persistent tiles (outside the timing repeat loop) ----
        w1s, w2s, b1s, b2s = [], [], [], []
        for e in range(E):
            w1 = wp.tile([128, 3, H1], BF16, tag=f"w1_{e}")
            nc.sync.dma_start(
                out=w1[:], in_=W1[e].rearrange("(kt kp) m -> kp kt m", kp=128)
            )
            w2 = wp.tile([128, 2, H2], BF16, tag=f"w2_{e}")
            nc.sync.dma_start(out=w2[:, 0, :], in_=W2[e][0:128, :])
            # kt1 weights duplicated at partition bases 0 and 64: matmul
            # requires lhsT.base_partition() == rhs.base_partition(), and
            # g1k1's two atom-halves live at partitions 0:64 / 64:128.
            nc.sync.dma_start(out=w2[0:64, 1, :], in_=W2[e][128:192, :])
            nc.sync.dma_start(out=w2[64:128, 1, :], in_=W2[e][128:192, :])
            w1s.append(w1)
            w2s.append(w2)
            if with_bias:
                b1 = wp.tile([1, H1], BF16, tag=f"b1_{e}")
                nc.sync.dma_start(out=b1[:], in_=B1[e : e + 1, :])
                b2 = wp.tile([1, H2], BF16, tag=f"b2_{e}")
                nc.sync.dma_start(out=b2[:], in_=B2[e : e + 1, :])
                b1s.append(b1)
                b2s.append(b2)
        ones = None
        if with_bias:
            ones = cp.tile([1, HB], BF16, tag="ones")
            nc.vector.memset(ones[:], 1.0)
        junkD = cp.tile([128, BLOCK], BF16, tag="junkD")
        SD = sp.tile([128, 2 * nb], F32, tag="SD")
        nc.vector.memset(SD[:], 0.0)

        loop_cm = tc.For_i(0, repeat, 1) if repeat > 1 else contextlib.nullcontext()
        with loop_cm:
            _emit_body(
                tc, xT3, w1s, w2s, b1s, b2s, ones, junkD, SD, with_bias, caps,
                xp, t1p, u1p, g1p, t2p, u2p, z1m0p, z1m1p, z2m0p, z2m1p,
            )
        nc.sync.dma_start(out=outS, in_=SD[:])


def _emit_body(
    tc, xT3, w1s, w2s, b1s, b2s, ones, junkD, SD, with_bias, caps,
    xp, t1p, u1p, g1p, t2p, u2p, z1m0p, z1m1p, z2m0p, z2m1p,
):
    nc = tc.nc
    ONLY = os.environ.get("ONLY", "full")
    nb = sum(caps) // BLOCK
    block_expert = []
    for e in range(E):
        block_expert += [e] * (caps[e] // BLOCK)

    PREFETCH = 2
    S = {}  # per-block pipeline state

    def dma_x(i):
        xa = xp.tile([128, 3, BLOCK], BF16, tag="xa")
        nc.sync.dma_start(
            out=xa[:], in_=xT3[:, :, i * BLOCK : (i + 1) * BLOCK]
        )
        return xa

    for i in range(min(PREFETCH, nb)):
        S[i] = {"xa": dma_x(i)}

    for i in range(nb + 2):
        # ---- stage P1(i-1): celu1 pointwise ----
        j = i - 1
        if 0 <= j < nb and ONLY in ("p1", "mm2", "full"):
            st = S[j]
            z1m0, z1m1 = st["z1m0"], st["z1m1"]
            t1 = t1p.tile([128, 1536], F32, tag="t1")
            # m1 first: unblocks mm1m1(i) (single-buffered Z1M1) earliest
            nc.scalar.activation(t1[:, 1024:1536], z1m1[:], AF.Exp)
            nc.scalar.activation(t1[:, 0:1024], z1m0[:], AF.Exp)
            u1 = u1p.tile([128, 1536], F32, tag="u1")
            g1k0 = g1p.tile([128, BLOCK], BF16, tag="g1k0")
            g1k1 = g1p.tile([128, HB], BF16, tag="g1k1")
            u1eng = nc.gpsimd if U1_POOL else nc.vector
            u1eng.tensor_scalar(
                out=u1[:, 1024:1536], in0=t1[:, 1024:1536],
                scalar1=-1.0, scalar2=0.0, op0=ALU.add, op1=ALU.min,
            )
            nc.vector.tensor_tensor(
                out=g1k1[:], in0=z1m1[:], in1=u1[:, 1024:1536], op=ALU.max
            )
            u1eng.tensor_scalar(
                out=u1[:, 0:1024], in0=t1[:, 0:1024],
                scalar1=-1.0, scalar2=0.0, op0=ALU.add, op1=ALU.min,
            )
            nc.vector.tensor_tensor(
                out=g1k0[:], in0=z1m0[:], in1=u1[:, 0:1024], op=ALU.max
            )
            st["g1k0"], st["g1k1"] = g1k0, g1k1

        # ---- stage MM1(i) ----
        if i < nb and ONLY != "dma":
            st = S[i]
            e = block_expert[i]
            w1 = w1s[e]
            xa = st["xa"]
            z1m0 = z1m0p.tile([128, BLOCK], F32, tag="z1m0")
            z1m1 = z1m1p.tile([128, HB], F32, tag="z1m1")
            last = 2 if not with_bias else -1
            for kt in range(3):
                for h in range(2):
                    nc.tensor.matmul(
                        z1m0[:, HB * h : HB * (h + 1)],
                        lhsT=w1[:, kt, 0:128],
                        rhs=xa[:, kt, HB * h : HB * (h + 1)],
                        start=(kt == 0), stop=(kt == last),
                    )
                for h in range(2):
                    nc.tensor.matmul(
                        z1m1[64 * h : 64 * (h + 1), :],
                        lhsT=w1[:, kt, 128:192],
                        rhs=xa[:, kt, HB * h : HB * (h + 1)],
                        start=(kt == 0), stop=(kt == last),
                    )
            if with_bias:
                b1 = b1s[e]
                for h in range(2):
                    nc.tensor.matmul(
                        z1m0[:, HB * h : HB * (h + 1)], lhsT=b1[:, 0:128],
                        rhs=ones[:], start=False, stop=True,
                    )
                for h in range(2):
                    nc.tensor.matmul(
                        z1m1[64 * h : 64 * (h + 1), :], lhsT=b1[:, 128:192],
                        rhs=ones[:], start=False, stop=True,
                    )
            st["z1m0"], st["z1m1"] = z1m0, z1m1

        # ---- stage MM2(i-1) ----
        if 0 <= j < nb and ONLY in ("mm2", "full"):
            st = S[j]
            e = block_expert[j]
            w2 = w2s[e]
            g1k0, g1k1 = st["g1k0"], st["g1k1"]
            z2m0 = z2m0p.tile([128, BLOCK], F32, tag="z2m0")
            z2m1 = z2m1p.tile([64, HB], F32, tag="z2m1")
            last_stop = not with_bias
            MM2P = int(os.environ.get("MM2P", "15"))
            _solo = MM2P != 15
            # kt0 m0 (LDW w2[:,0,0:128])
            for h in (range(2) if MM2P & 1 else []):
                nc.tensor.matmul(
                    z2m0[:, HB * h : HB * (h + 1)], lhsT=w2[:, 0, 0:128],
                    rhs=g1k0[:, HB * h : HB * (h + 1)], start=True, stop=(False or _solo),
                )
            # kt0 m1 (LDW w2[:,0,128:160]): atom-half h -> parts 32h,
            # cols 0:512 (k-row base 0; positions (0,0)/(0,32))
            for h in (range(2) if MM2P & 2 else []):
                nc.tensor.matmul(
                    z2m1[32 * h : 32 * h + 32, 0:HB],
                    lhsT=w2[:, 0, 128:160],
                    rhs=g1k0[:, HB * h : HB * (h + 1)],
                    start=True, stop=(False or _solo),
                )
            # kt1 m0; g1k1 parts: h half of atoms, lhsT at matching base
            for h in (range(2) if MM2P & 4 else []):
                nc.tensor.matmul(
                    z2m0[:, HB * h : HB * (h + 1)],
                    lhsT=w2[64 * h : 64 * h + 64, 1, 0:128],
                    rhs=g1k1[64 * h : 64 * (h + 1), :],
                    start=(False or _solo), stop=last_stop,
                )
            # kt1 m1: positions (0,0) h0 / (64,32) h1 — the safe diagonal
            for h in (range(2) if MM2P & 8 else []):
                nc.tensor.matmul(
                    z2m1[32 * h : 32 * h + 32, 0:HB],
                    lhsT=w2[64 * h : 64 * h + 64, 1, 128:160],
                    rhs=g1k1[64 * h : 64 * (h + 1), 0:HB],
                    start=(False or _solo), stop=last_stop,
                )
            if with_bias:
                b2 = b2s[e]
                for h in range(2):
                    nc.tensor.matmul(
                        z2m0[:, HB * h : HB * (h + 1)], lhsT=b2[:, 0:128],
                        rhs=ones[:], start=False, stop=True,
                    )
                for h in range(2):
                    nc.tensor.matmul(
                        z2m1[32 * h : 32 * h + 32, 0:HB],
                        lhsT=b2[:, 128:160],
                        rhs=ones[:], start=False, stop=True,
                    )
            st["z2m0"], st["z2m1"] = z2m0, z2m1

        # ---- stage P2(i-1): layer-2 pointwise + fused celu sums ----
        k = i - 1
        if 0 <= k < nb and ONLY == "full":
            st = S.pop(k)
            z2m0, z2m1 = st["z2m0"], st["z2m1"]
            t2 = t2p.tile([128, 1536], F32, tag="t2")
            u2 = u2p.tile([128, 1536], F32, tag="u2")
            nc.scalar.activation(t2[:, 0:1024], z2m0[:], AF.Exp)
            nc.scalar.activation(t2[0:64, 1024:1536], z2m1[:], AF.Exp)
            if UN2M0 == "act":
                # un2m0 = relu(1 - t2) = -u2 on ACT; sign folded into STT
                nc.scalar.activation(
                    u2[:, 0:1024], t2[:, 0:1024], AF.Relu, bias=1.0, scale=-1.0
                )
                nc.vector.scalar_tensor_tensor(
                    out=junkD[:], in0=u2[:, 0:1024], scalar=-1.0, in1=z2m0[:],
                    op0=ALU.mult, op1=ALU.max,
                    accum_out=SD[:, 2 * k : 2 * k + 1],
                )
            else:
                nc.vector.tensor_scalar(
                    out=u2[:, 0:1024], in0=t2[:, 0:1024],
                    scalar1=-1.0, scalar2=0.0, op0=ALU.add, op1=ALU.min,
                )
                nc.vector.scalar_tensor_tensor(
                    out=junkD[:], in0=u2[:, 0:1024], scalar=1.0, in1=z2m0[:],
                    op0=ALU.mult, op1=ALU.max,
                    accum_out=SD[:, 2 * k : 2 * k + 1],
                )
            u2m1eng = nc.gpsimd if U2M1 == "pool" else nc.vector
            u2m1eng.tensor_scalar(
                out=u2[0:64, 1024:1536], in0=t2[0:64, 1024:1536],
                scalar1=-1.0, scalar2=0.0, op0=ALU.add, op1=ALU.min,
            )
            nc.vector.scalar_tensor_tensor(
                out=junkD[0:64, 0:512], in0=u2[0:64, 1024:1536], scalar=1.0,
                in1=z2m1[:], op0=ALU.mult, op1=ALU.max,
                accum_out=SD[0:64, 2 * k + 1 : 2 * k + 2],
            )

        # ---- prefetch ----
        nxt = i + PREFETCH
        if nxt < nb:
            S[nxt] = {"xa": dma_x(nxt)}


_GRAPH_CACHE = {}


def _get_graph(with_bias: bool, caps):
    key = (with_bias, tuple(caps))
    if key not in _GRAPH_CACHE:
        _GRAPH_CACHE[key] = _build_graph(with_bias, caps)
    return _GRAPH_CACHE[key]


def _celu64(v):
    return np.where(v > 0, v, np.expm1(np.minimum(v, 0.0)))


def prepare_in_maps(aev_inputs, atom_types, W1, b1, W2, b2, W3, b3):
    """Host routing: build per-core input maps + metadata for decode."""
    import ml_dtypes

    ndt = ml_dtypes.bfloat16
    aev = np.asarray(aev_inputs, dtype=np.float32)
    types = np.asarray(atom_types).astype(np.int64)
    W1f = np.asarray(W1, dtype=np.float32)
    b1 = np.asarray(b1, dtype=np.float32)
    W2f = np.asarray(W2, dtype=np.float32)
    b2 = np.asarray(b2, dtype=np.float32)
    W3f = np.asarray(W3, dtype=np.float32)
    b3 = np.asarray(b3, dtype=np.float32)
    W1b = np.ascontiguousarray(W1f.astype(ndt))
    W2b = np.ascontiguousarray(W2f.astype(ndt))

    with_bias = bool(np.any(b1) or np.any(b2))

    order = np.argsort(types, kind="stable")
    sorted_types = types[order]
    bounds = np.searchsorted(sorted_types, np.arange(E + 1))
    type_lists = [order[bounds[e] : bounds[e + 1]] for e in range(E)]

    SHED_MAX = 192
    slices = [[None] * E for _ in range(N_CORES)]
    n_real = np.zeros((N_CORES, E), dtype=np.int64)
    shed = []
    caps = []
    for e in range(E):
        lst = type_lists[e]
        counts = [
            ((len(lst) * (c + 1)) // N_CORES) - ((len(lst) * c) // N_CORES)
            for c in range(N_CORES)
        ]
        mx = max(counts)
        rem = mx % BLOCK
        if 0 < rem <= SHED_MAX:
            cap_e = (mx // BLOCK) * BLOCK
        else:
            cap_e = -(-mx // BLOCK) * BLOCK
        caps.append(cap_e)
        for c in range(N_CORES):
            lo = (len(lst) * c) // N_CORES
            hi = (len(lst) * (c + 1)) // N_CORES
            take = min(hi - lo, cap_e)
            slices[c][e] = lst[lo : lo + take]
            shed.append(lst[lo + take : hi])
            n_real[c, e] = take
    shed = np.concatenate(shed) if shed else np.zeros(0, dtype=np.int64)
    caps = tuple(caps)
    offs = np.cumsum([0] + list(caps))

    shed_energy = 0.0
    if len(shed):
        xs = aev[shed].astype(np.float64)
        ts_ = types[shed]
        for e in range(E):
            m = ts_ == e
            if not m.any():
                continue
            h = _celu64(xs[m] @ W1f[e].astype(np.float64) + b1[e].astype(np.float64))
            h = _celu64(h @ W2f[e].astype(np.float64) + b2[e].astype(np.float64))
            y = h @ W3f[e].astype(np.float64)[:, 0] + float(b3[e][0])
            shed_energy += float(y.sum())

    in_maps = []
    for c in range(N_CORES):
        xcT = np.zeros((IN_DIM, int(offs[-1])), dtype=ndt)
        for e in range(E):
            idx = slices[c][e]
            xcT[:, int(offs[e]) : int(offs[e]) + len(idx)] = aev[idx].T.astype(ndt)
        m = {"xT": xcT, "W1": W1b, "W2": W2b}
        if with_bias:
            m["B1"] = np.ascontiguousarray(b1.astype(ndt))
            m["B2"] = np.ascontiguousarray(b2.astype(ndt))
        in_maps.append(m)
    return in_maps, n_real, with_bias, (b1, W2f, b2, W3f, b3, shed_energy), caps


def postprocess(results, n_real, wdata, caps):
    """Decode per-block sum columns -> per-expert per-dim sums -> energy."""
    b1, W2f, b2, W3f, b3, shed_energy = wdata
    nb = sum(caps) // BLOCK
    block_expert = []
    for e in range(E):
        block_expert += [e] * (caps[e] // BLOCK)

    S = np.zeros((E, H2), dtype=np.float64)  # sum of celu(z2) per expert/dim
    for c in range(N_CORES):
        D = np.asarray(results[c]["outS"], dtype=np.float64)  # [128, 2nb]
        for b in range(nb):
            e = block_expert[b]
            S[e, 0:128] += D[:, 2 * b]
            # m1 col: partitions 0:64, dim = 128 + p % 32
            S[e, 128:160] += D[0:64, 2 * b + 1].reshape(2, 32).sum(axis=0)

    total = shed_energy
    counts_e = n_real.sum(axis=0)
    pads_e = np.array([N_CORES * caps[e] - counts_e[e] for e in range(E)])
    for e in range(E):
        w3 = W3f[e].astype(np.float64)[:, 0]
        total += float(w3 @ S[e])
        total += float(counts_e[e]) * float(b3[e][0])
        if pads_e[e]:
            # device pads contribute celu(z2_0) per dim; subtract (f64 model)
            h1 = _celu64(b1[e].astype(np.float64))
            z2_0 = h1 @ W2f[e].astype(np.float64) + b2[e].astype(np.float64)
            total -= float(pads_e[e]) * float(w3 @ _celu64(z2_0))
    return np.asarray(total, dtype=np.float32)


def kernel(aev_inputs, atom_types, W1, b1, W2, b2, W3, b3):
    in_maps, n_real, with_bias, wdata, caps = prepare_in_maps(
        aev_inputs, atom_types, W1, b1, W2, b2, W3, b3
    )
    nc = _get_graph(with_bias, caps)
    results = bass_utils.run_bass_kernel_spmd(
        nc, in_maps, core_ids=list(range(N_CORES))
    ).results
    return postprocess(results, n_real, wdata, caps)


# revision 15
# speedup vs baseline: 1.0599x; 1.0599x over previous
"""ANI-style MoE routing kernel for 8 Trainium2 NeuronCores — v2.

Strategy (data-parallel + host routing):
  - Host: sort atoms by type, split each type's list evenly across 8 cores,
    build per-core per-expert contiguous batches padded to per-expert
    capacities (multiples of 1024).  Batches are feature-major bf16
    [384, sum(caps)].  Small overflows (<= SHED_MAX atoms) are computed
    exactly on the host in f64 instead of paying a whole device block.
  - Device (per core), per 1024-atom block, dim-major:
      mm1 (PE): z1 = W1^T x.  H1=192 -> m0 [128,1024] + m1 [64,1024]
        folded to [128,512] (two 512-atom column halves stacked on
        partitions 0/64) so pointwise ops touch zero garbage lanes.
      celu1: t1 = exp(z1) [ACT]; u1 = min(t1-1,0) [DVE ts 2x]; g1 =
        max(z1,u1) -> bf16 [DVE tt].
      mm2 (PE): z2 = W2^T g1.  H2=160 -> m0 [128,1024] + m1 [32,1024]
        folded to [64,512] (quarters at partition 0/32 x column halves).
      layer 3 is linear and the final output is a scalar, so NO mm3:
        per-dim sums of celu(z2) suffice.  celu(z2) = max(z2, u2) with
        u2 = min(exp(z2)-1, 0), so one DVE scalar_tensor_tensor /
        tensor_tensor_reduce pass with accum_out yields the per-block
        per-dim sums directly.  The u2 helper runs on ACT as
        relu(1-exp(z2)) (= -u2, sign folded into the STT) for m0 and on
        DVE for m1, balancing engine load.  Sum columns land in an SBUF
        tile, DMA'd out once at the end.
  - Host: S_e[dim] = sum of block columns; energy = sum_e w3[e]^T S_e in
    f64, + b3*counts + shed energies.

Zero-bias (always true for this problem's init) makes padding rows
self-cancelling: z=0 -> celu contribution exactly 0.  The general-bias
path adds per-layer bias matmuls and corrects pads on the host.
"""

import os
import sys

import numpy as np

try:
    import concourse.bass as bass  # noqa: F401
except ImportError:  # pragma: no cover
    sys.path.insert(0, "/opt/trn_rl_repo")
    import concourse.bass as bass  # noqa: F401

import concourse.mybir as mybir
import concourse.tile as tile
from concourse import bacc
from concourse import bass_utils

IN_DIM = 384
H1 = 192
H2 = 160
E = 4
N_CORES = 8
N_ATOMS = 262144

BLOCK = 1024
HB = 512  # half block

F32 = mybir.dt.float32
BF16 = mybir.dt.bfloat16
AF = mybir.ActivationFunctionType
ALU = mybir.AluOpType

# engine assignment knobs (A/B-testable)
UN2M0 = os.environ.get("UN2M0", "act")  # act | dve
U2M1 = os.environ.get("U2M1", "dve")  # dve | pool
U1_POOL = os.environ.get("U1_POOL", "0") == "1"


def _build_graph(with_bias: bool, caps, repeat: int = 1):
    nc = bacc.Bacc(
        "TRN2",
        target_bir_lowering=False,
        debug=False,
        enable_asserts=False,
        num_devices=N_CORES,
    )
    total_cap = sum(caps)
    nb = total_cap // BLOCK
    xT = nc.dram_tensor("xT", [IN_DIM, total_cap], BF16, kind="ExternalInput").ap()
    W1 = nc.dram_tensor("W1", [E, IN_DIM, H1], BF16, kind="ExternalInput").ap()
    W2 = nc.dram_tensor("W2", [E, H1, H2], BF16, kind="ExternalInput").ap()
    B1 = B2 = None
    if with_bias:
        B1 = nc.dram_tensor("B1", [E, H1], BF16, kind="ExternalInput").ap()
        B2 = nc.dram_tensor("B2", [E, H2], BF16, kind="ExternalInput").ap()
    outS = nc.dram_tensor("outS", [128, 2 * nb], F32, kind="ExternalOutput").ap()

    with tile.TileContext(nc) as tc:
        _emit(tc, xT, W1, W2, B1, B2, outS, with_bias, caps, repeat)
    nc.compile()
    return nc


def _emit(tc, xT, W1, W2, B1, B2, outS, with_bias, caps, repeat=1):
    import contextlib

    nc = tc.nc
    xT3 = xT.rearrange("(kt kp) n -> kp kt n", kp=128)  # [128, 3, total]
    nb = sum(caps) // BLOCK

    with (
        tc.tile_pool(name="wpool", bufs=1) as wp,
        tc.tile_pool(name="xpool", bufs=int(os.environ.get("X_BUFS", "3"))) as xp,
        tc.tile_pool(name="t1pool", bufs=2) as t1p,
        tc.tile_pool(name="u1pool", bufs=2) as u1p,
        tc.tile_pool(name="g1pool", bufs=2) as g1p,
        tc.tile_pool(name="t2pool", bufs=2) as t2p,
        tc.tile_pool(name="u2pool", bufs=2) as u2p,
        tc.tile_pool(name="cpool", bufs=1) as cp,
        tc.tile_pool(name="spool", bufs=1) as sp,
        tc.tile_pool(name="z1m0p", bufs=2, space="PSUM") as z1m0p,
        tc.tile_pool(name="z1m1p", bufs=1, space="PSUM") as z1m1p,
        tc.tile_pool(name="z2m0p", bufs=1, space="PSUM") as z2m0p,
        tc.tile_pool(name="z2m1p", bufs=1, space="PSUM") as z2m1p,
    ):
        # ---- persistent tiles (outside the timing repeat loop) ----
        w1s, w2s, b1s, b2s = [], [], [], []
        for e in range(E):
            w1 = wp.tile([128, 3, H1], BF16, tag=f"w1_{e}")
            nc.sync.dma_start(
                out=w1[:], in_=W1[e].rearrange("(kt kp) m -> kp kt m", kp=128)
            )
            w2 = wp.tile([128, 2, H2], BF16, tag=f"w2_{e}")
            nc.sync.dma_start(out=w2[:, 0, :], in_=W2[e][0:128, :])
            # kt1 weights duplicated at partition bases 0 and 64: matmul
            # requires lhsT.base_partition() == rhs.base_partition(), and
            # g1k1's two atom-halves live at partitions 0:64 / 64:128.
            nc.sync.dma_start(out=w2[0:64, 1, :], in_=W2[e][128:192, :])
            nc.sync.dma_start(out=w2[64:128, 1, :], in_=W2[e][128:192, :])
            w1s.append(w1)
            w2s.append(w2)
            if with_bias:
                b1 = wp.tile([1, H1], BF16, tag=f"b1_{e}")
                nc.sync.dma_start(out=b1[:], in_=B1[e : e + 1, :])
                b2 = wp.tile([1, H2], BF16, tag=f"b2_{e}")
                nc.sync.dma_start(out=b2[:], in_=B2[e : e + 1, :])
                b1s.append(b1)
                b2s.append(b2)
        ones = None
        if with_bias:
            ones = cp.tile([1, HB], BF16, tag="ones")
            nc.vector.memset(ones[:], 1.0)
        junkD = cp.tile([128, BLOCK], BF16, tag="junkD")
        SD = sp.tile([128, 2 * nb], F32, tag="SD")
        nc.vector.memset(SD[:], 0.0)

        loop_cm = tc.For_i(0, repeat, 1) if repeat > 1 else contextlib.nullcontext()
        with loop_cm:
            _emit_body(
                tc, xT3, w1s, w2s, b1s, b2s, ones, junkD, SD, with_bias, caps,
                xp, t1p, u1p, g1p, t2p, u2p, z1m0p, z1m1p, z2m0p, z2m1p,
            )
        nc.sync.dma_start(out=outS, in_=SD[:])


def _emit_body(
    tc, xT3, w1s, w2s, b1s, b2s, ones, junkD, SD, with_bias, caps,
    xp, t1p, u1p, g1p, t2p, u2p, z1m0p, z1m1p, z2m0p, z2m1p,
):
    nc = tc.nc
    ONLY = os.environ.get("ONLY", "full")
    nb = sum(caps) // BLOCK
    block_expert = []
    for e in range(E):
        block_expert += [e] * (caps[e] // BLOCK)

    PREFETCH = 2
    S = {}  # per-block pipeline state

    def dma_x(i):
        xa = xp.tile([128, 3, BLOCK], BF16, tag="xa")
        nc.sync.dma_start(
            out=xa[:], in_=xT3[:, :, i * BLOCK : (i + 1) * BLOCK]
        )
        return xa

    for i in range(min(PREFETCH, nb)):
        S[i] = {"xa": dma_x(i)}

    for i in range(nb + 2):
        # ---- stage P1(i-1): celu1 pointwise ----
        j = i - 1
        if 0 <= j < nb and ONLY in ("p1", "mm2", "full"):
            st = S[j]
            z1m0, z1m1 = st["z1m0"], st["z1m1"]
            t1 = t1p.tile([128, 1536], F32, tag="t1")
            # m1 first: unblocks mm1m1(i) (single-buffered Z1M1) earliest
            nc.scalar.activation(t1[:, 1024:1536], z1m1[:], AF.Exp)
            nc.scalar.activation(t1[:, 0:1024], z1m0[:], AF.Exp)
            u1 = u1p.tile([128, 1536], F32, tag="u1")
            g1k0 = g1p.tile([128, BLOCK], BF16, tag="g1k0")
            g1k1 = g1p.tile([128, HB], BF16, tag="g1k1")
            u1eng = nc.gpsimd if U1_POOL else nc.vector
            u1eng.tensor_scalar(
                out=u1[:, 1024:1536], in0=t1[:, 1024:1536],
                scalar1=-1.0, scalar2=0.0, op0=ALU.add, op1=ALU.min,
            )
            nc.vector.tensor_tensor(
                out=g1k1[:], in0=z1m1[:], in1=u1[:, 1024:1536], op=ALU.max
            )
            u1eng.tensor_scalar(
                out=u1[:, 0:1024], in0=t1[:, 0:1024],
                scalar1=-1.0, scalar2=0.0, op0=ALU.add, op1=ALU.min,
            )
            nc.vector.tensor_tensor(
                out=g1k0[:], in0=z1m0[:], in1=u1[:, 0:1024], op=ALU.max
            )
            st["g1k0"], st["g1k1"] = g1k0, g1k1

        # ---- stage P2(i-2): layer-2 pointwise + fused celu sums ----
        k = i - 2
        if 0 <= k < nb and ONLY == "full":
            st = S.pop(k)
            z2m0, z2m1 = st["z2m0"], st["z2m1"]
            t2 = t2p.tile([128, 1536], F32, tag="t2")
            u2 = u2p.tile([128, 1536], F32, tag="u2")
            nc.scalar.activation(t2[:, 0:1024], z2m0[:], AF.Exp)
            nc.scalar.activation(t2[0:64, 1024:1536], z2m1[:], AF.Exp)
            if UN2M0 == "act":
                # un2m0 = relu(1 - t2) = -u2 on ACT; sign folded into STT
                nc.scalar.activation(
                    u2[:, 0:1024], t2[:, 0:1024], AF.Relu, bias=1.0, scale=-1.0
                )
                nc.vector.scalar_tensor_tensor(
                    out=junkD[:], in0=u2[:, 0:1024], scalar=-1.0, in1=z2m0[:],
                    op0=ALU.mult, op1=ALU.max,
                    accum_out=SD[:, 2 * k : 2 * k + 1],
                )
            else:
                nc.vector.tensor_scalar(
                    out=u2[:, 0:1024], in0=t2[:, 0:1024],
                    scalar1=-1.0, scalar2=0.0, op0=ALU.add, op1=ALU.min,
                )
                nc.vector.scalar_tensor_tensor(
                    out=junkD[:], in0=u2[:, 0:1024], scalar=1.0, in1=z2m0[:],
                    op0=ALU.mult, op1=ALU.max,
                    accum_out=SD[:, 2 * k : 2 * k + 1],
                )
            u2m1eng = nc.gpsimd if U2M1 == "pool" else nc.vector
            u2m1eng.tensor_scalar(
                out=u2[0:64, 1024:1536], in0=t2[0:64, 1024:1536],
                scalar1=-1.0, scalar2=0.0, op0=ALU.add, op1=ALU.min,
            )
            nc.vector.scalar_tensor_tensor(
                out=junkD[0:64, 0:512], in0=u2[0:64, 1024:1536], scalar=1.0,
                in1=z2m1[:], op0=ALU.mult, op1=ALU.max,
                accum_out=SD[0:64, 2 * k + 1 : 2 * k + 2],
            )

        # ---- stage MM1(i) ----
        if i < nb and ONLY != "dma":
            st = S[i]
            e = block_expert[i]
            w1 = w1s[e]
            xa = st["xa"]
            z1m0 = z1m0p.tile([128, BLOCK], F32, tag="z1m0")
            z1m1 = z1m1p.tile([128, HB], F32, tag="z1m1")
            last = 2 if not with_bias else -1
            for kt in range(3):
                for h in range(2):
                    nc.tensor.matmul(
                        z1m0[:, HB * h : HB * (h + 1)],
                        lhsT=w1[:, kt, 0:128],
                        rhs=xa[:, kt, HB * h : HB * (h + 1)],
                        start=(kt == 0), stop=(kt == last),
                    )
                for h in range(2):
                    nc.tensor.matmul(
                        z1m1[64 * h : 64 * (h + 1), :],
                        lhsT=w1[:, kt, 128:192],
                        rhs=xa[:, kt, HB * h : HB * (h + 1)],
                        start=(kt == 0), stop=(kt == last),
                    )
            if with_bias:
                b1 = b1s[e]
                for h in range(2):
                    nc.tensor.matmul(
                        z1m0[:, HB * h : HB * (h + 1)], lhsT=b1[:, 0:128],
                        rhs=ones[:], start=False, stop=True,
                    )
                for h in range(2):
                    nc.tensor.matmul(
                        z1m1[64 * h : 64 * (h + 1), :], lhsT=b1[:, 128:192],
                        rhs=ones[:], start=False, stop=True,
                    )
            st["z1m0"], st["z1m1"] = z1m0, z1m1

        # ---- stage MM2(i-1) ----
        if 0 <= j < nb and ONLY in ("mm2", "full"):
            st = S[j]
            e = block_expert[j]
            w2 = w2s[e]
            g1k0, g1k1 = st["g1k0"], st["g1k1"]
            z2m0 = z2m0p.tile([128, BLOCK], F32, tag="z2m0")
            z2m1 = z2m1p.tile([64, HB], F32, tag="z2m1")
            last_stop = not with_bias
            MM2P = int(os.environ.get("MM2P", "15"))
            _solo = MM2P != 15
            # kt0 m0 (LDW w2[:,0,0:128])
            for h in (range(2) if MM2P & 1 else []):
                nc.tensor.matmul(
                    z2m0[:, HB * h : HB * (h + 1)], lhsT=w2[:, 0, 0:128],
                    rhs=g1k0[:, HB * h : HB * (h + 1)], start=True, stop=(False or _solo),
                )
            # kt0 m1 (LDW w2[:,0,128:160]): atom-half h -> parts 32h,
            # cols 0:512 (k-row base 0; positions (0,0)/(0,32))
            for h in (range(2) if MM2P & 2 else []):
                nc.tensor.matmul(
                    z2m1[32 * h : 32 * h + 32, 0:HB],
                    lhsT=w2[:, 0, 128:160],
                    rhs=g1k0[:, HB * h : HB * (h + 1)],
                    start=True, stop=(False or _solo),
                )
            # kt1 m0; g1k1 parts: h half of atoms, lhsT at matching base
            for h in (range(2) if MM2P & 4 else []):
                nc.tensor.matmul(
                    z2m0[:, HB * h : HB * (h + 1)],
                    lhsT=w2[64 * h : 64 * h + 64, 1, 0:128],
                    rhs=g1k1[64 * h : 64 * (h + 1), :],
                    start=(False or _solo), stop=last_stop,
                )
            # kt1 m1: positions (0,0) h0 / (64,32) h1 — the safe diagonal
            for h in (range(2) if MM2P & 8 else []):
                nc.tensor.matmul(
                    z2m1[32 * h : 32 * h + 32, 0:HB],
                    lhsT=w2[64 * h : 64 * h + 64, 1, 128:160],
                    rhs=g1k1[64 * h : 64 * (h + 1), 0:HB],
                    start=(False or _solo), stop=last_stop,
                )
            if with_bias:
                b2 = b2s[e]
                for h in range(2):
                    nc.tensor.matmul(
                        z2m0[:, HB * h : HB * (h + 1)], lhsT=b2[:, 0:128],
                        rhs=ones[:], start=False, stop=True,
                    )
                for h in range(2):
                    nc.tensor.matmul(
                        z2m1[32 * h : 32 * h + 32, 0:HB],
                        lhsT=b2[:, 128:160],
                        rhs=ones[:], start=False, stop=True,
                    )
            st["z2m0"], st["z2m1"] = z2m0, z2m1

        # ---- prefetch ----
        nxt = i + PREFETCH
        if nxt < nb:
            S[nxt] = {"xa": dma_x(nxt)}


_GRAPH_CACHE = {}


def _get_graph(with_bias: bool, caps):
    key = (with_bias, tuple(caps))
    if key not in _GRAPH_CACHE:
        _GRAPH_CACHE[key] = _build_graph(with_bias, caps)
    return _GRAPH_CACHE[key]


def _celu64(v):
    return np.where(v > 0, v, np.expm1(np.minimum(v, 0.0)))


def prepare_in_maps(aev_inputs, atom_types, W1, b1, W2, b2, W3, b3):
    """Host routing: build per-core input maps + metadata for decode."""
    import ml_dtypes

    ndt = ml_dtypes.bfloat16
    aev = np.asarray(aev_inputs, dtype=np.float32)
    types = np.asarray(atom_types).astype(np.int64)
    W1f = np.asarray(W1, dtype=np.float32)
    b1 = np.asarray(b1, dtype=np.float32)
    W2f = np.asarray(W2, dtype=np.float32)
    b2 = np.asarray(b2, dtype=np.float32)
    W3f = np.asarray(W3, dtype=np.float32)
    b3 = np.asarray(b3, dtype=np.float32)
    W1b = np.ascontiguousarray(W1f.astype(ndt))
    W2b = np.ascontiguousarray(W2f.astype(ndt))

    with_bias = bool(np.any(b1) or np.any(b2))

    order = np.argsort(types, kind="stable")
    sorted_types = types[order]
    bounds = np.searchsorted(sorted_types, np.arange(E + 1))
    type_lists = [order[bounds[e] : bounds[e + 1]] for e in range(E)]

    SHED_MAX = 192
    slices = [[None] * E for _ in range(N_CORES)]
    n_real = np.zeros((N_CORES, E), dtype=np.int64)
    shed = []
    caps = []
    for e in range(E):
        lst = type_lists[e]
        counts = [
            ((len(lst) * (c + 1)) // N_CORES) - ((len(lst) * c) // N_CORES)
            for c in range(N_CORES)
        ]
        mx = max(counts)
        rem = mx % BLOCK
        if 0 < rem <= SHED_MAX:
            cap_e = (mx // BLOCK) * BLOCK
        else:
            cap_e = -(-mx // BLOCK) * BLOCK
        caps.append(cap_e)
        for c in range(N_CORES):
            lo = (len(lst) * c) // N_CORES
            hi = (len(lst) * (c + 1)) // N_CORES
            take = min(hi - lo, cap_e)
            slices[c][e] = lst[lo : lo + take]
            shed.append(lst[lo + take : hi])
            n_real[c, e] = take
    shed = np.concatenate(shed) if shed else np.zeros(0, dtype=np.int64)
    caps = tuple(caps)
    offs = np.cumsum([0] + list(caps))

    shed_energy = 0.0
    if len(shed):
        xs = aev[shed].astype(np.float64)
        ts_ = types[shed]
        for e in range(E):
            m = ts_ == e
            if not m.any():
                continue
            h = _celu64(xs[m] @ W1f[e].astype(np.float64) + b1[e].astype(np.float64))
            h = _celu64(h @ W2f[e].astype(np.float64) + b2[e].astype(np.float64))
            y = h @ W3f[e].astype(np.float64)[:, 0] + float(b3[e][0])
            shed_energy += float(y.sum())

    in_maps = []
    for c in range(N_CORES):
        xcT = np.zeros((IN_DIM, int(offs[-1])), dtype=ndt)
        for e in range(E):
            idx = slices[c][e]
            xcT[:, int(offs[e]) : int(offs[e]) + len(idx)] = aev[idx].T.astype(ndt)
        m = {"xT": xcT, "W1": W1b, "W2": W2b}
        if with_bias:
            m["B1"] = np.ascontiguousarray(b1.astype(ndt))
            m["B2"] = np.ascontiguousarray(b2.astype(ndt))
        in_maps.append(m)
    return in_maps, n_real, with_bias, (b1, W2f, b2, W3f, b3, shed_energy), caps


def postprocess(results, n_real, wdata, caps):
    """Decode per-block sum columns -> per-expert per-dim sums -> energy."""
    b1, W2f, b2, W3f, b3, shed_energy = wdata
    nb = sum(caps) // BLOCK
    block_expert = []
    for e in range(E):
        block_expert += [e] * (caps[e] // BLOCK)

    S = np.zeros((E, H2), dtype=np.float64)  # sum of celu(z2) per expert/dim
    for c in range(N_CORES):
        D = np.asarray(results[c]["outS"], dtype=np.float64)  # [128, 2nb]
        for b in range(nb):
            e = block_expert[b]
            S[e, 0:128] += D[:, 2 * b]
            # m1 col: partitions 0:64, dim = 128 + p % 32
            S[e, 128:160] += D[0:64, 2 * b + 1].reshape(2, 32).sum(axis=0)

    total = shed_energy
    counts_e = n_real.sum(axis=0)
    pads_e = np.array([N_CORES * caps[e] - counts_e[e] for e in range(E)])
    for e in range(E):
        w3 = W3f[e].astype(np.float64)[:, 0]
        total += float(w3 @ S[e])
        total += float(counts_e[e]) * float(b3[e][0])
        if pads_e[e]:
            # device pads contribute celu(z2_0) per dim; subtract (f64 model)
            h1 = _celu64(b1[e].astype(np.float64))
            z2_0 = h1 @ W2f[e].astype(np.float64) + b2[e].astype(np.float64)
            total -= float(pads_e[e]) * float(w3 @ _celu64(z2_0))
    return np.asarray(total, dtype=np.float32)


def kernel(aev_inputs, atom_types, W1, b1, W2, b2, W3, b3):
    in_maps, n_real, with_bias, wdata, caps = prepare_in_maps(
        aev_inputs, atom_types, W1, b1, W2, b2, W3, b3
    )
    nc = _get_graph(with_bias, caps)
    results = bass_utils.run_bass_kernel_spmd(
        nc, in_maps, core_ids=list(range(N_CORES))
    ).results
    return postprocess(results, n_real, wdata, caps)
